# revision 49
# baseline (speedup 1.0000x reference)
"""Multi-head attention (B=2, T=2048, D=1024, H=16, causal) on 8 TRN2 NeuronCores.

Sharding (tensor-parallel heads + token-parallel epilogue):
  - Core c owns heads (2c, 2c+1) -> a 128-wide slice of the QKV output dim.
  - All matmul operands are bf16 (fp32 PSUM accumulation).
  - Fully-interleaved single-pass schedule: attention for batch-0 q-block 0
    starts as soon as the first projection pair lands; every remaining
    QKV-projection slab, V re-layout block and output projection is emitted
    as PE "filler" work inside the attention k-loop so the TensorE stream
    stays dense (no >3us PE gaps -> no HAM re-throttle).
  - DMA queues are segregated by role so latency-critical transfers never
    queue behind bulk:
      scalar: startup bulk only (wq/wk/wv + first x slab) - idle before exps
      sync:   small latency chain (softmax-denominator bounce, a2a scatter,
              ctxg gather, output writes) + one startup slab
      gpsimd: collective triggers + mid-kernel bulk (later x slabs, wo, bo)
  - Attention: streaming 128-wide key blocks; transposed score tiles
    S^T [k, q] for both heads in one [128, 1024] PSUM pair; causal columns
    trimmed at the diagonal; 128-wide partial triangle masked by a DVE
    multiply with a 0/1 triangle; softmax denominator accumulated as ctx
    row 64 via a ones column in V.  The k-loop is software-pipelined (ctx of
    block k issues after scores of block k+1).
  - ctx accumulates in ONE [128, 1024] PSUM tile (head0 cols 0:512, head1
    cols 512:1024); softmax normalization is all on-chip: denominator row
    -> one SBUF->SBUF DMA hop to partition 0 (HW custom ops ignore AP base
    partitions) -> reciprocal_approx_fast -> gpsimd partition_broadcast ->
    one wide DVE multiply (~5us chain vs ~25us for a DRAM-bounce chain).
  - a2a is per q-block pair (4 chunks of [8, 128, 128] bf16; 8 smaller
    chunks oversubscribe the serialized CC stream at ~21us/op).  Output
    projections run per chunk once its collective has soaked >= 4
    q-blocks; tail ops pre-gather their ctxg right after the NEXT chunk's
    doorbell so only the last collective is ever waited on.
"""

import numpy as np
import ml_dtypes

import concourse.bacc as bacc
import concourse.bass as bass
import concourse.mybir as mybir
import concourse.tile as tile
from concourse import bass_utils
from concourse.bass import ts

D = 1024
H = 16
DK = D // H  # 64
NCORES = 8
HPC = H // NCORES  # heads per core = 2
DSL = HPC * DK  # per-core QKV output slice = 128
P = 128
QBLK = 512
KBLK = 128
DA = DK + 1  # 65: head dim + ones column (softmax denominator row)
KPQ = QBLK // KBLK  # key blocks per q-block = 4

F32 = mybir.dt.float32
F32R = mybir.dt.float32r
BF16 = mybir.dt.bfloat16
EXP = mybir.ActivationFunctionType.Exp
NP_BF16 = ml_dtypes.bfloat16


def build_nc(B=2, T=2048, debug_taps=False):
    """Build the SPMD Bass module (identical program on all 8 cores)."""
    NTOK = B * T
    KO = D // P  # 8 contraction chunks
    NKB = T // KBLK  # key blocks per batch
    NQB = T // QBLK  # query blocks per batch
    G = B * NQB  # total q-blocks
    NCHUNK = G // 2  # a2a chunks (one per q-block pair)
    TPB = 2 * QBLK // NCORES  # tokens per core per a2a chunk = 128
    NOP = NCHUNK  # output projections (one per chunk, 128 tokens)
    NSLAB = NTOK // QBLK  # x token slabs
    NPAIR = NSLAB // 2
    JPB = T // P  # v_nat 128-token blocks per batch
    NJ = NTOK // P

    nc = bacc.Bacc("TRN2", target_bir_lowering=False, debug=False,
                   num_devices=NCORES)

    # ---- DRAM I/O ------------------------------------------------------
    xT_d = nc.dram_tensor("xT", [P, NSLAB, KO, QBLK], BF16, kind="ExternalInput")
    wqT_d = nc.dram_tensor("wqT", [P, KO, DSL], BF16, kind="ExternalInput")
    wkT_d = nc.dram_tensor("wkT", [P, KO, DSL], BF16, kind="ExternalInput")
    wvT_d = nc.dram_tensor("wvT", [P, KO, DSL], BF16, kind="ExternalInput")
    woT_d = nc.dram_tensor("woT", [P, KO, D], BF16, kind="ExternalInput")
    bq_d = nc.dram_tensor("bq", [DSL, 1], F32, kind="ExternalInput")
    bk_d = nc.dram_tensor("bk", [DSL, 1], F32, kind="ExternalInput")
    bv_d = nc.dram_tensor("bv", [DSL, 1], F32, kind="ExternalInput")
    bo_d = nc.dram_tensor("bo", [D], F32, kind="ExternalInput")
    mask_d = nc.dram_tensor("mask", [P, KBLK], BF16, kind="ExternalInput")
    ident_d = nc.dram_tensor("ident", [P, P], BF16, kind="ExternalInput")
    identr_d = nc.dram_tensor("identr", [P, P], F32R, kind="ExternalInput")
    ones_d = nc.dram_tensor("ones", [P, NJ], BF16, kind="ExternalInput")
    out_d = nc.dram_tensor("out", [NOP * P, D], F32, kind="ExternalOutput")

    with tile.TileContext(nc) as tc:
        with (
            tc.tile_pool(name="consts", bufs=1) as consts,
            tc.tile_pool(name="acts", bufs=1) as acts,
            tc.tile_pool(name="xin", bufs=4) as xin,
            tc.tile_pool(name="attn", bufs=2) as attn_pool,
            tc.tile_pool(name="small", bufs=2) as small,
            tc.tile_pool(name="outg", bufs=4) as outg,
            tc.tile_pool(name="outp", bufs=2) as outp,
            tc.tile_pool(name="psA", bufs=2, space="PSUM") as psA,
            tc.tile_pool(name="psC", bufs=2, space="PSUM") as psC,
            tc.tile_pool(name="cc", bufs=max(G, 2), space="DRAM") as ccp,
        ):
            # ---- small constants on sync (tiny) ------------------------
            ident_sb = consts.tile([P, P], BF16, tag="ident")
            nc.sync.dma_start(ident_sb[:], ident_d.ap())
            bq_sb = consts.tile([P, 1], F32, tag="bq")
            bk_sb = consts.tile([P, 1], F32, tag="bk")
            bv_sb = consts.tile([P, 1], F32, tag="bv")
            nc.sync.dma_start(bq_sb[:], bq_d.ap())
            nc.sync.dma_start(bk_sb[:], bk_d.ap())
            nc.sync.dma_start(bv_sb[:], bv_d.ap())
            mask_sb = consts.tile([P, KBLK], BF16, tag="mask")
            nc.sync.dma_start(mask_sb[:], mask_d.ap())
            identr_sb = consts.tile([P, P], F32R, tag="identr")
            nc.sync.dma_start(identr_sb[:], identr_d.ap())

            # ---- bulk startup: wq + pair-0 slabs split across the two
            # startup queues so the first projections stream ASAP --------
            wq_sb = consts.tile([P, KO, DSL], BF16, tag="wq")
            wk_sb = consts.tile([P, KO, DSL], BF16, tag="wk")
            wv_sb = consts.tile([P, KO, DSL], BF16, tag="wv")
            nc.scalar.dma_start(wq_sb[:], wqT_d.ap())

            xts = {}  # slab index -> sbuf tile

            def load_slab(s, dma, halves=1):
                t = xin.tile([P, KO, QBLK], BF16, tag="xt", name=f"xt{s}")
                if halves == 2:
                    dma.dma_start(t[:, 0:KO // 2], xT_d.ap()[:, s, 0:KO // 2])
                    dma.dma_start(t[:, KO // 2:], xT_d.ap()[:, s, KO // 2:])
                else:
                    dma.dma_start(t[:], xT_d.ap()[:, s])
                xts[s] = t
                return t

            load_slab(0, nc.scalar, halves=2)
            load_slab(1, nc.sync, halves=2)
            nc.scalar.dma_start(wk_sb[:], wkT_d.ap())
            nc.scalar.dma_start(wv_sb[:], wvT_d.ap())
            # pair-1 slabs on gpsimd: no collectives queued there yet, and
            # the triggers fire immediately (xin bufs are free).  Later
            # slabs are deferred into the main loop so their buffer-free
            # waits never block the gpsimd latency chain.
            for s in range(2, min(4, NSLAB)):
                load_slab(s, nc.gpsimd)

            def warmup(n, moving=None):
                """Dummy matmuls to keep the PE HAM clock-gate at 2.4GHz
                through unavoidable waits (startup, final collective)."""
                warm = psA.tile([P, 2 * QBLK], F32, tag="sp", name="warm")
                mv = ident_sb[:] if moving is None else moving
                for _ in range(n):
                    nc.tensor.matmul(warm[:, 0:mv.shape[-1]], ident_sb[:],
                                     mv, start=True, stop=True)

            warmup(12)

            qT = acts.tile([P, NTOK], BF16, tag="qT")
            kT = acts.tile([P, NTOK], BF16, tag="kT")
            vT = acts.tile([P, NTOK], F32R, tag="vT")
            v_nat = acts.tile([P, NJ, 2 * DA], BF16, tag="v_nat")
            nc.sync.dma_start(v_nat[:, :, DK], ones_d.ap())
            nc.sync.dma_start(v_nat[:, :, DA + DK], ones_d.ap())

            def proj_slab(w_sb, b_sb, dst, s):
                """One projection for one 512-token slab: 8 accumulating
                matmuls into a 1-bank PSUM tile + bias epilogue on DVE.
                Self-contained so it can be dropped anywhere as PE filler."""
                xt = xts[s]
                ps = psA.tile([P, QBLK], F32, tag="sp", name="ps")
                for ko in range(KO):
                    nc.tensor.matmul(ps[:], w_sb[:, ko], xt[:, ko],
                                     start=(ko == 0), stop=(ko == KO - 1))
                nc.vector.tensor_scalar_add(dst[:, ts(s, QBLK)], ps[:],
                                            b_sb[:, 0:1])

            def proj_pair(i):
                for w_sb, b_sb, dst in ((wq_sb, bq_sb, qT),
                                        (wk_sb, bk_sb, kT),
                                        (wv_sb, bv_sb, vT)):
                    proj_slab(w_sb, b_sb, dst, 2 * i)
                    proj_slab(w_sb, b_sb, dst, 2 * i + 1)

            def v_nat_block(j):
                """Transpose one [128,128] vT tile into v_nat (both heads);
                ones columns stay intact."""
                ptf = psA.tile([P, P], F32R, tag="sp", name="ptf")
                nc.tensor.transpose(ptf[:], vT[:, ts(j, P)], identr_sb[:])
                nc.vector.tensor_copy(v_nat[:, j, 0:DK], ptf[:, 0:DK])
                nc.vector.tensor_copy(v_nat[:, j, DA:DA + DK], ptf[:, DK:P])

            a2a_in = [ccp.tile([NCORES, P, TPB], BF16, tag="a2a_in",
                               name=f"a2a_in{k}") for k in range(NCHUNK)]
            a2a_out = [ccp.tile([NCORES, P, TPB], BF16, tag="a2a_out",
                                name=f"a2a_out{k}") for k in range(NCHUNK)]

            def collective(k):
                nc.gpsimd.collective_compute(
                    "AllToAll",
                    mybir.AluOpType.bypass,
                    replica_groups=[list(range(NCORES))],
                    ins=[a2a_in[k][:].opt()],
                    outs=[a2a_out[k][:].opt()],
                )

            # wide tail constants, DMA'd on gpsimd early (after the slab
            # triggers) so they never contend with the latency chain
            wo_sb = consts.tile([P, KO, D], BF16, tag="wo")
            bo_sb = consts.tile([P, D], F32, tag="bo")

            def load_wo():
                nc.gpsimd.dma_start(wo_sb[:], woT_d.ap())
                nc.gpsimd.dma_start(
                    bo_sb[:], bo_d.ap()[None, :].to_broadcast((P, D)))

            def outproj_gather(p):
                """Issue the ctxg gather DMA for output projection p (chunk
                p's collective must have completed)."""
                ctxg = outg.tile([P, KO, TPB], BF16, tag="ctxg",
                                 name=f"ctxg{p}")
                nc.sync.dma_start(ctxg[:],
                                  a2a_out[p][:].rearrange("j p t -> p j t"))
                return ctxg

            def outproj_half(p, ctxg, dh):
                """Half output projection (512 of 1024 output features) for
                chunk pair p: 8 matmuls into a 1-bank PSUM tile."""
                po = psA.tile([P, QBLK], F32, tag="sp", name=f"po{p}_{dh}")
                dsl_ = ts(dh, QBLK)
                for ko in range(KO):
                    nc.tensor.matmul(po[:], ctxg[:, ko], wo_sb[:, ko, dsl_],
                                     start=(ko == 0), stop=(ko == KO - 1))
                o_sb = outp.tile([P, QBLK], F32, tag="o_sb", name=f"o{p}_{dh}")
                nc.vector.tensor_add(o_sb[:], po[:], bo_sb[:, dsl_])
                nc.sync.dma_start(out_d.ap()[ts(p, P), dsl_], o_sb[:])

            def attention_qblock(b, qi, fill):
                g = b * NQB + qi
                q0 = (b * NQB + qi) * QBLK
                nkb = (qi + 1) * KPQ
                C = psC.tile([P, 2 * QBLK], F32, tag="ctx", name="C")

                def emit_ctx(pend):
                    ap_, jjp, lo, st, sp = pend
                    nc.tensor.matmul(C[0:DA, lo:QBLK], v_nat[:, jjp, 0:DA],
                                     ap_[:, lo:QBLK], start=st, stop=sp)
                    nc.tensor.matmul(C[0:DA, QBLK + lo:],
                                     v_nat[:, jjp, DA:2 * DA],
                                     ap_[:, QBLK + lo:], start=st, stop=sp)

                nfill = len(fill)
                done = 0
                pend = None
                for ki in range(nkb):
                    k_sl = ts(b * NKB + ki, KBLK)
                    jj = b * JPB + ki
                    doff = ki * KBLK - qi * QBLK
                    diag = doff >= 0
                    lo = max(doff, 0)
                    sp_t = psA.tile([P, 2 * QBLK], F32, tag="sp", name="sp_t")
                    nc.tensor.matmul(sp_t[:, lo:QBLK],
                                     kT[0:DK, k_sl],
                                     qT[0:DK, q0 + lo:q0 + QBLK],
                                     start=True, stop=True,
                                     tile_position=(0, 0))
                    nc.tensor.matmul(sp_t[:, QBLK + lo:],
                                     kT[DK:P, k_sl],
                                     qT[DK:P, q0 + lo:q0 + QBLK],
                                     start=True, stop=True,
                                     tile_position=(64, 0))
                    a_p = attn_pool.tile([P, 2 * QBLK], BF16, tag="ap",
                                         name="a_p")
                    if lo:
                        src = sp_t[:].rearrange("p (h q) -> p h q", h=2)[:, :, lo:]
                        dst = a_p[:].rearrange("p (h q) -> p h q", h=2)[:, :, lo:]
                    else:
                        src, dst = sp_t[:], a_p[:]
                    nc.scalar.activation(dst, src, EXP)
                    if diag:
                        nc.vector.tensor_mul(a_p[:, lo:lo + KBLK],
                                             a_p[:, lo:lo + KBLK], mask_sb[:])
                        nc.vector.tensor_mul(
                            a_p[:, QBLK + lo:QBLK + lo + KBLK],
                            a_p[:, QBLK + lo:QBLK + lo + KBLK], mask_sb[:])
                    if pend is not None:
                        emit_ctx(pend)
                    pend = (a_p, jj, lo, ki == 0, ki == nkb - 1)
                    # evenly interleave the PE filler work (never after the
                    # final ctx matmul -- the latency chain must start the
                    # moment C is complete)
                    if ki < nkb - 1:
                        while done < ((ki + 1) * nfill) // nkb:
                            fill[done]()
                            done += 1
                emit_ctx(pend)

                # softmax normalize, one DMA hop: extract the denominator
                # row (PSUM partition 64) to SBUF, DMA it to partition 0
                # (the custom DVE/GpSimd ops operate on absolute partition
                # 0 in hardware), approx-reciprocal, partition-broadcast on
                # the idle GpSimd engine, one wide DVE multiply, scatter.
                den = small.tile([P, 2 * QBLK], F32, tag="den")
                nc.vector.tensor_copy(den[DK:DA, :], C[DK:DA, :])
                den0 = small.tile([1, 2 * QBLK], F32, tag="den0")
                nc.sync.dma_start(den0[0:1, :], den[DK:DA, :])
                rec = small.tile([1, 2 * QBLK], F32, tag="rec")
                nc.vector.reciprocal_approx_fast(rec[0:1, :], den0[0:1, :])
                rb_sb = small.tile([P, 2 * QBLK], F32, tag="rb_sb")
                nc.gpsimd.partition_broadcast(rb_sb[0:DK, :], rec[0:1, :],
                                              channels=DK)
                ctx_sb = small.tile([P, 2 * QBLK], BF16, tag="ctx_sb")
                nc.vector.tensor_mul(ctx_sb[0:DK, :], C[0:DK, :],
                                     rb_sb[0:DK, :])
                # scatter both heads: dst core d = (g%2)*4 + s owns tokens
                # [d*TPB, (d+1)*TPB) of chunk g//2.
                chunk = g // 2
                dsl_s = ts(g % 2, QBLK // TPB)
                nc.sync.dma_start(
                    a2a_in[chunk][dsl_s, 0:DK].rearrange("s p t -> p s t"),
                    ctx_sb[0:DK, 0:QBLK].rearrange("p (s t) -> p s t",
                                                   s=QBLK // TPB))
                nc.sync.dma_start(
                    a2a_in[chunk][dsl_s, DK:P].rearrange("s p t -> p s t"),
                    ctx_sb[0:DK, QBLK:].rearrange("p (s t) -> p s t",
                                                  s=QBLK // TPB))
                # leftover fillers after the chain is in flight
                while done < nfill:
                    fill[done]()
                    done += 1

            # ---- interleave plan ---------------------------------------
            # fillers[g] = PE work units emitted inside q-block g's k-loop
            fillers = [[] for _ in range(G)]

            # proj pairs >=1: 6 slab-proj units each, placed in the two
            # q-blocks before the first q-block that touches their tokens
            for p_ in range(1, NPAIR):
                toks0 = p_ * 2 * QBLK
                b_p = toks0 // T
                qi_first = (toks0 % T) // QBLK
                g_need = b_p * NQB + qi_first
                span = [gg for gg in (g_need - 2, g_need - 1) if gg >= 0]
                units = []
                # slab-major so the first span q-block only needs slab 2p
                for s in (2 * p_, 2 * p_ + 1):
                    for w_sb, b_sb, dst in ((wq_sb, bq_sb, qT),
                                            (wk_sb, bk_sb, kT),
                                            (wv_sb, bv_sb, vT)):
                        units.append(
                            lambda w=w_sb, bb=b_sb, dd=dst, ss=s:
                            proj_slab(w, bb, dd, ss))
                if len(span) == 1:
                    fillers[span[0]] += units
                else:
                    fillers[span[0]] += units[:3]
                    fillers[span[1]] += units[3:]

            # v_nat blocks: j needed by q-block (b, jloc//KPQ); emit one
            # q-block earlier (prologue covers the first KPQ blocks)
            vnat_pro = []
            for j in range(NJ):
                b_j = j // JPB
                g_need = b_j * NQB + (j % JPB) // KPQ
                g_slot = g_need - 1
                if g_slot < 0:
                    vnat_pro.append(j)
                else:
                    fillers[g_slot].append(lambda jj=j: v_nat_block(jj))

            # output projections: op p consumes chunk p, whose collective
            # is triggered at the end of q-block 2p+1.  Collectives take
            # 20-50us to COMPLETE (barrier + serialized CC stream), and a
            # gather emitted too early head-blocks the sync queue (the
            # scheduler hoists it ahead of later scatters) -- so only
            # schedule an op as filler with >= 4 q-blocks of soak.
            ctxg_tiles = {}

            def outproj_compute(p):
                ctxg = ctxg_tiles.pop(p, None)
                if ctxg is None:
                    ctxg = outproj_gather(p)
                outproj_half(p, ctxg, 0)
                outproj_half(p, ctxg, 1)

            tail_ps = []
            for p_ in range(NOP):
                g_slot = 2 * p_ + 6
                if g_slot < G:
                    fillers[g_slot].append(lambda pp=p_: outproj_compute(pp))
                else:
                    tail_ps.append(p_)
            # tail ops: pre-issue each gather right after the NEXT chunk's
            # doorbell (its own collective completed a full chunk ago), so
            # the tail's sync queue only ever waits on the LAST collective
            pregather = {p_ + 1: p_ for p_ in tail_ps if p_ + 1 < NCHUNK}

            # later x slabs + the wide tail weights on gpsimd, staggered
            # so their buffer-free waits never delay a chunk's broadcast
            gp_bulk = {}
            for s in range(4, NSLAB):
                gp_bulk.setdefault(s - 4, []).append(
                    lambda ss=s: load_slab(ss, nc.gpsimd))
            gp_bulk.setdefault(min(1, G - 2), []).append(load_wo)

            # ---- emission ----------------------------------------------
            proj_pair(0)
            for j in vnat_pro:
                v_nat_block(j)

            g = 0
            for b in range(B):
                for qi in range(NQB):
                    attention_qblock(b, qi, fillers[g])
                    if g % 2 == 1:
                        k = g // 2
                        collective(k)
                        if k in pregather:
                            ctxg_tiles[pregather[k]] = outproj_gather(
                                pregather[k])
                    for fn in gp_bulk.get(g, []):
                        fn()
                    g += 1

            # tail: the last chunk's collective is in flight; its input is
            # pre-gathered for all but the last op, so the reserved output
            # projections + a short warmup cover the collective latency
            assert tail_ps
            for p_ in tail_ps[:-1]:
                outproj_compute(p_)
            warmup(10, moving=qT[:, 0:QBLK])
            outproj_compute(tail_ps[-1])

    nc.compile()
    return nc


_NC_CACHE = {}


def _get_nc(B, T):
    key = (B, T)
    if key not in _NC_CACHE:
        _NC_CACHE[key] = build_nc(B, T)
    return _NC_CACHE[key]


def make_in_maps(x, Wq, bq, Wk, bk, Wv, bv, Wo, bo):
    B, T, _ = x.shape
    NTOK = B * T
    NSLAB = NTOK // QBLK
    KO = D // P
    x = np.asarray(x, np.float32)
    # [D, NTOK] -> [p, slab, ko, t]: one contiguous 8KB DMA descriptor per
    # partition per token slab.
    xT = x.reshape(NTOK, D).T  # [D, NTOK]
    xT_t = np.ascontiguousarray(
        xT.reshape(KO, P, NSLAB, QBLK).transpose(1, 2, 0, 3)).astype(NP_BF16)

    def wtile(W):
        # [D, M] -> [p, ko, m] so each partition's row is contiguous
        wt = np.asarray(W, np.float32)
        return np.ascontiguousarray(
            wt.reshape(KO, P, -1).transpose(1, 0, 2)).astype(NP_BF16)

    woT = wtile(np.asarray(Wo, np.float32).T)
    bo = np.asarray(bo, np.float32)
    # 128-wide causal 0/1 triangle for the diagonal partial columns
    keep = np.arange(KBLK)[None, :] >= np.arange(P)[:, None]
    mask = np.where(keep, 1.0, 0.0).astype(NP_BF16)
    ident = np.eye(P, dtype=NP_BF16)
    ones = np.ones((P, NTOK // P), NP_BF16)
    in_maps = []
    for c in range(NCORES):
        sl = slice(DSL * c, DSL * (c + 1))
        in_maps.append({
            "xT": xT_t,
            "wqT": wtile(np.asarray(Wq, np.float32)[sl].T * 0.125),
            "wkT": wtile(np.asarray(Wk, np.float32)[sl].T),
            "wvT": wtile(np.asarray(Wv, np.float32)[sl].T),
            "woT": woT,
            "bq": (np.asarray(bq, np.float32)[sl] * 0.125).reshape(DSL, 1),
            "bk": np.asarray(bk, np.float32)[sl].reshape(DSL, 1),
            "bv": np.asarray(bv, np.float32)[sl].reshape(DSL, 1),
            "bo": bo,
            "mask": mask,
            "ident": ident,
            "identr": np.eye(P, dtype=np.float32),
            "ones": ones,
        })
    return in_maps


def unshard(res_c, B, T):
    """res_c: [NCORES, NCHUNK*128, D] core-major outputs -> [B, T, D].

    Core c's rows are [chunk, 128] with chunk k covering tokens
    [k*1024 + c*128, k*1024 + (c+1)*128)."""
    NCHUNK = B * (T // QBLK) // 2
    TPB = 2 * QBLK // NCORES
    out = res_c.reshape(NCORES, NCHUNK, TPB, D).transpose(1, 0, 2, 3)
    return np.ascontiguousarray(out.reshape(B, T, D))


LAST_RESULTS = None


def kernel(x, Wq, bq, Wk, bk, Wv, bv, Wo, bo, trace=False, trace_cores=None):
    global LAST_RESULTS
    B, T, _ = x.shape
    nc = _get_nc(B, T)
    in_maps = make_in_maps(x, Wq, bq, Wk, bk, Wv, bv, Wo, bo)
    kw = {}
    if trace:
        kw = dict(trace=True, trace_cores=trace_cores)
    res = bass_utils.run_bass_kernel_spmd(nc, in_maps,
                                          core_ids=list(range(NCORES)), **kw)
    LAST_RESULTS = res
    res_c = np.stack([res.results[c]["out"] for c in range(NCORES)], axis=0)
    return unshard(res_c, B, T)


# revision 50
# speedup vs baseline: 1.0229x; 1.0229x over previous
"""Multi-head attention (B=2, T=2048, D=1024, H=16, causal) on 8 TRN2 NeuronCores.

Sharding (tensor-parallel heads + token-parallel epilogue):
  - Core c owns heads (2c, 2c+1) -> a 128-wide slice of the QKV output dim.
  - All matmul operands are bf16 (fp32 PSUM accumulation).
  - Fully-interleaved single-pass schedule: attention for batch-0 q-block 0
    starts as soon as the first projection pair lands; every remaining
    QKV-projection slab, V re-layout block and output projection is emitted
    as PE "filler" work inside the attention k-loop so the TensorE stream
    stays dense (no >3us PE gaps -> no HAM re-throttle).
  - DMA queues are segregated by role so latency-critical transfers never
    queue behind bulk:
      scalar: startup bulk only (wq/wk/wv + first x slab) - idle before exps
      sync:   small latency chain (softmax-denominator bounce, a2a scatter,
              ctxg gather, output writes) + one startup slab
      gpsimd: collective triggers + mid-kernel bulk (later x slabs, wo, bo)
  - Attention: streaming 128-wide key blocks; transposed score tiles
    S^T [k, q] for both heads in one [128, 1024] PSUM pair; causal columns
    trimmed at the diagonal; 128-wide partial triangle masked by a DVE
    multiply with a 0/1 triangle; softmax denominator accumulated as ctx
    row 64 via a ones column in V.  The k-loop is software-pipelined (ctx of
    block k issues after scores of block k+1).
  - ctx accumulates in ONE [128, 1024] PSUM tile (head0 cols 0:512, head1
    cols 512:1024); softmax normalization is all on-chip: denominator row
    -> one SBUF->SBUF DMA hop to partition 0 (HW custom ops ignore AP base
    partitions) -> reciprocal_approx_fast -> gpsimd partition_broadcast ->
    one wide DVE multiply (~5us chain vs ~25us for a DRAM-bounce chain).
  - a2a is per q-block pair (4 chunks of [8, 128, 128] bf16; 8 smaller
    chunks oversubscribe the serialized CC stream at ~21us/op).  Output
    projections run per chunk once its collective has soaked >= 4
    q-blocks; tail ops pre-gather their ctxg right after the NEXT chunk's
    doorbell so only the last collective is ever waited on.
"""

import numpy as np
import ml_dtypes

import concourse.bacc as bacc
import concourse.bass as bass
import concourse.mybir as mybir
import concourse.tile as tile
from concourse import bass_utils
from concourse.bass import ts

D = 1024
H = 16
DK = D // H  # 64
NCORES = 8
HPC = H // NCORES  # heads per core = 2
DSL = HPC * DK  # per-core QKV output slice = 128
P = 128
QBLK = 512
KBLK = 128
DA = DK + 1  # 65: head dim + ones column (softmax denominator row)
KPQ = QBLK // KBLK  # key blocks per q-block = 4

F32 = mybir.dt.float32
F32R = mybir.dt.float32r
BF16 = mybir.dt.bfloat16
EXP = mybir.ActivationFunctionType.Exp
NP_BF16 = ml_dtypes.bfloat16


def build_nc(B=2, T=2048, debug_taps=False):
    """Build the SPMD Bass module (identical program on all 8 cores)."""
    NTOK = B * T
    KO = D // P  # 8 contraction chunks
    NKB = T // KBLK  # key blocks per batch
    NQB = T // QBLK  # query blocks per batch
    G = B * NQB  # total q-blocks
    NCHUNK = G // 2  # a2a chunks (one per q-block pair)
    TPB = 2 * QBLK // NCORES  # tokens per core per a2a chunk = 128
    NOP = NCHUNK  # output projections (one per chunk, 128 tokens)
    NSLAB = NTOK // QBLK  # x token slabs
    NPAIR = NSLAB // 2
    JPB = T // P  # v_nat 128-token blocks per batch
    NJ = NTOK // P

    nc = bacc.Bacc("TRN2", target_bir_lowering=False, debug=False,
                   num_devices=NCORES)

    # ---- DRAM I/O ------------------------------------------------------
    xT_d = nc.dram_tensor("xT", [P, NSLAB, KO, QBLK], BF16, kind="ExternalInput")
    wqT_d = nc.dram_tensor("wqT", [P, KO, DSL], BF16, kind="ExternalInput")
    wkT_d = nc.dram_tensor("wkT", [P, KO, DSL], BF16, kind="ExternalInput")
    wvT_d = nc.dram_tensor("wvT", [P, KO, DSL], BF16, kind="ExternalInput")
    woT_d = nc.dram_tensor("woT", [P, KO, D], BF16, kind="ExternalInput")
    bq_d = nc.dram_tensor("bq", [DSL, 1], F32, kind="ExternalInput")
    bk_d = nc.dram_tensor("bk", [DSL, 1], F32, kind="ExternalInput")
    bv_d = nc.dram_tensor("bv", [DSL, 1], F32, kind="ExternalInput")
    bo_d = nc.dram_tensor("bo", [D], F32, kind="ExternalInput")
    mask_d = nc.dram_tensor("mask", [P, KBLK], BF16, kind="ExternalInput")
    ident_d = nc.dram_tensor("ident", [P, P], BF16, kind="ExternalInput")
    identr_d = nc.dram_tensor("identr", [P, P], F32R, kind="ExternalInput")
    ones_d = nc.dram_tensor("ones", [P, NJ], BF16, kind="ExternalInput")
    out_d = nc.dram_tensor("out", [NOP * P, D], F32, kind="ExternalOutput")

    with tile.TileContext(nc) as tc:
        with (
            tc.tile_pool(name="consts", bufs=1) as consts,
            tc.tile_pool(name="acts", bufs=1) as acts,
            tc.tile_pool(name="xin", bufs=4) as xin,
            tc.tile_pool(name="attn", bufs=2) as attn_pool,
            tc.tile_pool(name="small", bufs=2) as small,
            tc.tile_pool(name="outg", bufs=4) as outg,
            tc.tile_pool(name="outp", bufs=2) as outp,
            tc.tile_pool(name="psA", bufs=2, space="PSUM") as psA,
            tc.tile_pool(name="psC", bufs=2, space="PSUM") as psC,
            tc.tile_pool(name="cc", bufs=max(G, 2), space="DRAM") as ccp,
        ):
            # ---- small constants on sync (tiny) ------------------------
            ident_sb = consts.tile([P, P], BF16, tag="ident")
            nc.sync.dma_start(ident_sb[:], ident_d.ap())
            bq_sb = consts.tile([P, 1], F32, tag="bq")
            bk_sb = consts.tile([P, 1], F32, tag="bk")
            bv_sb = consts.tile([P, 1], F32, tag="bv")
            nc.sync.dma_start(bq_sb[:], bq_d.ap())
            nc.sync.dma_start(bk_sb[:], bk_d.ap())
            nc.sync.dma_start(bv_sb[:], bv_d.ap())
            mask_sb = consts.tile([P, KBLK], BF16, tag="mask")
            nc.sync.dma_start(mask_sb[:], mask_d.ap())
            identr_sb = consts.tile([P, P], F32R, tag="identr")
            nc.sync.dma_start(identr_sb[:], identr_d.ap())

            # ---- bulk startup: wq + pair-0 slabs split across the two
            # startup queues so the first projections stream ASAP --------
            wq_sb = consts.tile([P, KO, DSL], BF16, tag="wq")
            wk_sb = consts.tile([P, KO, DSL], BF16, tag="wk")
            wv_sb = consts.tile([P, KO, DSL], BF16, tag="wv")
            nc.scalar.dma_start(wq_sb[:], wqT_d.ap())

            xts = {}  # slab index -> sbuf tile

            def load_slab(s, dma, halves=1):
                t = xin.tile([P, KO, QBLK], BF16, tag="xt", name=f"xt{s}")
                if halves == 2:
                    dma.dma_start(t[:, 0:KO // 2], xT_d.ap()[:, s, 0:KO // 2])
                    dma.dma_start(t[:, KO // 2:], xT_d.ap()[:, s, KO // 2:])
                else:
                    dma.dma_start(t[:], xT_d.ap()[:, s])
                xts[s] = t
                return t

            # slab 0 in quarters with the weight loads interleaved so the
            # q/k/v projections can each start the moment their first
            # contraction chunks land (the old [wq, xt0, wk, wv] order
            # stalled the k-projection until ~11us)
            xt0 = xin.tile([P, KO, QBLK], BF16, tag="xt", name="xt0")
            xts[0] = xt0
            nc.scalar.dma_start(xt0[:, 0:2], xT_d.ap()[:, 0, 0:2])
            nc.scalar.dma_start(wk_sb[:], wkT_d.ap())
            nc.scalar.dma_start(xt0[:, 2:4], xT_d.ap()[:, 0, 2:4])
            nc.scalar.dma_start(wv_sb[:], wvT_d.ap())
            nc.scalar.dma_start(xt0[:, 4:8], xT_d.ap()[:, 0, 4:8])
            load_slab(1, nc.sync, halves=2)
            # pair-1 slabs on gpsimd: no collectives queued there yet, and
            # the triggers fire immediately (xin bufs are free).  Later
            # slabs are deferred into the main loop so their buffer-free
            # waits never block the gpsimd latency chain.
            for s in range(2, min(4, NSLAB)):
                load_slab(s, nc.gpsimd)

            def warmup(n, moving=None):
                """Dummy matmuls to keep the PE HAM clock-gate at 2.4GHz
                through unavoidable waits (startup, final collective)."""
                warm = psA.tile([P, 2 * QBLK], F32, tag="sp", name="warm")
                mv = ident_sb[:] if moving is None else moving
                for _ in range(n):
                    nc.tensor.matmul(warm[:, 0:mv.shape[-1]], ident_sb[:],
                                     mv, start=True, stop=True)

            warmup(12)

            qT = acts.tile([P, NTOK], BF16, tag="qT")
            kT = acts.tile([P, NTOK], BF16, tag="kT")
            vT = acts.tile([P, NTOK], F32R, tag="vT")
            v_nat = acts.tile([P, NJ, 2 * DA], BF16, tag="v_nat")
            nc.sync.dma_start(v_nat[:, :, DK], ones_d.ap())
            nc.sync.dma_start(v_nat[:, :, DA + DK], ones_d.ap())

            def proj_slab(w_sb, b_sb, dst, s):
                """One projection for one 512-token slab: 8 accumulating
                matmuls into a 1-bank PSUM tile + bias epilogue on DVE.
                Self-contained so it can be dropped anywhere as PE filler."""
                xt = xts[s]
                ps = psA.tile([P, QBLK], F32, tag="sp", name="ps")
                for ko in range(KO):
                    nc.tensor.matmul(ps[:], w_sb[:, ko], xt[:, ko],
                                     start=(ko == 0), stop=(ko == KO - 1))
                nc.vector.tensor_scalar_add(dst[:, ts(s, QBLK)], ps[:],
                                            b_sb[:, 0:1])

            def proj_pair(i):
                for w_sb, b_sb, dst in ((wq_sb, bq_sb, qT),
                                        (wk_sb, bk_sb, kT),
                                        (wv_sb, bv_sb, vT)):
                    proj_slab(w_sb, b_sb, dst, 2 * i)
                    proj_slab(w_sb, b_sb, dst, 2 * i + 1)

            def v_nat_block(j):
                """Transpose one [128,128] vT tile into v_nat (both heads);
                ones columns stay intact."""
                ptf = psA.tile([P, P], F32R, tag="sp", name="ptf")
                nc.tensor.transpose(ptf[:], vT[:, ts(j, P)], identr_sb[:])
                nc.vector.tensor_copy(v_nat[:, j, 0:DK], ptf[:, 0:DK])
                nc.vector.tensor_copy(v_nat[:, j, DA:DA + DK], ptf[:, DK:P])

            a2a_in = [ccp.tile([NCORES, P, TPB], BF16, tag="a2a_in",
                               name=f"a2a_in{k}") for k in range(NCHUNK)]
            a2a_out = [ccp.tile([NCORES, P, TPB], BF16, tag="a2a_out",
                                name=f"a2a_out{k}") for k in range(NCHUNK)]

            def collective(k):
                nc.gpsimd.collective_compute(
                    "AllToAll",
                    mybir.AluOpType.bypass,
                    replica_groups=[list(range(NCORES))],
                    ins=[a2a_in[k][:].opt()],
                    outs=[a2a_out[k][:].opt()],
                )

            # wide tail constants, DMA'd on gpsimd early (after the slab
            # triggers) so they never contend with the latency chain
            wo_sb = consts.tile([P, KO, D], BF16, tag="wo")
            bo_sb = consts.tile([P, D], F32, tag="bo")

            def load_wo():
                nc.gpsimd.dma_start(wo_sb[:], woT_d.ap())
                nc.gpsimd.dma_start(
                    bo_sb[:], bo_d.ap()[None, :].to_broadcast((P, D)))

            def outproj_gather(p):
                """Issue the ctxg gather DMA for output projection p (chunk
                p's collective must have completed)."""
                ctxg = outg.tile([P, KO, TPB], BF16, tag="ctxg",
                                 name=f"ctxg{p}")
                nc.sync.dma_start(ctxg[:],
                                  a2a_out[p][:].rearrange("j p t -> p j t"))
                return ctxg

            def outproj_half(p, ctxg, dh):
                """Half output projection (512 of 1024 output features) for
                chunk pair p: 8 matmuls into a 1-bank PSUM tile."""
                po = psA.tile([P, QBLK], F32, tag="sp", name=f"po{p}_{dh}")
                dsl_ = ts(dh, QBLK)
                for ko in range(KO):
                    nc.tensor.matmul(po[:], ctxg[:, ko], wo_sb[:, ko, dsl_],
                                     start=(ko == 0), stop=(ko == KO - 1))
                o_sb = outp.tile([P, QBLK], F32, tag="o_sb", name=f"o{p}_{dh}")
                nc.vector.tensor_add(o_sb[:], po[:], bo_sb[:, dsl_])
                nc.sync.dma_start(out_d.ap()[ts(p, P), dsl_], o_sb[:])

            def attention_qblock(b, qi, fill):
                g = b * NQB + qi
                q0 = (b * NQB + qi) * QBLK
                nkb = (qi + 1) * KPQ
                C = psC.tile([P, 2 * QBLK], F32, tag="ctx", name="C")

                def emit_ctx(pend):
                    ap_, jjp, lo, st, sp = pend
                    nc.tensor.matmul(C[0:DA, lo:QBLK], v_nat[:, jjp, 0:DA],
                                     ap_[:, lo:QBLK], start=st, stop=sp)
                    nc.tensor.matmul(C[0:DA, QBLK + lo:],
                                     v_nat[:, jjp, DA:2 * DA],
                                     ap_[:, QBLK + lo:], start=st, stop=sp)

                nfill = len(fill)
                done = 0
                pend = None
                for ki in range(nkb):
                    k_sl = ts(b * NKB + ki, KBLK)
                    jj = b * JPB + ki
                    doff = ki * KBLK - qi * QBLK
                    diag = doff >= 0
                    lo = max(doff, 0)
                    sp_t = psA.tile([P, 2 * QBLK], F32, tag="sp", name="sp_t")
                    nc.tensor.matmul(sp_t[:, lo:QBLK],
                                     kT[0:DK, k_sl],
                                     qT[0:DK, q0 + lo:q0 + QBLK],
                                     start=True, stop=True,
                                     tile_position=(0, 0))
                    nc.tensor.matmul(sp_t[:, QBLK + lo:],
                                     kT[DK:P, k_sl],
                                     qT[DK:P, q0 + lo:q0 + QBLK],
                                     start=True, stop=True,
                                     tile_position=(64, 0))
                    a_p = attn_pool.tile([P, 2 * QBLK], BF16, tag="ap",
                                         name="a_p")
                    if lo:
                        src = sp_t[:].rearrange("p (h q) -> p h q", h=2)[:, :, lo:]
                        dst = a_p[:].rearrange("p (h q) -> p h q", h=2)[:, :, lo:]
                    else:
                        src, dst = sp_t[:], a_p[:]
                    nc.scalar.activation(dst, src, EXP)
                    if diag:
                        nc.vector.tensor_mul(a_p[:, lo:lo + KBLK],
                                             a_p[:, lo:lo + KBLK], mask_sb[:])
                        nc.vector.tensor_mul(
                            a_p[:, QBLK + lo:QBLK + lo + KBLK],
                            a_p[:, QBLK + lo:QBLK + lo + KBLK], mask_sb[:])
                    if pend is not None:
                        emit_ctx(pend)
                    pend = (a_p, jj, lo, ki == 0, ki == nkb - 1)
                    # evenly interleave the PE filler work (never after the
                    # final ctx matmul -- the latency chain must start the
                    # moment C is complete)
                    if ki < nkb - 1:
                        while done < ((ki + 1) * nfill) // nkb:
                            fill[done]()
                            done += 1
                emit_ctx(pend)

                # softmax normalize, one DMA hop: extract the denominator
                # row (PSUM partition 64) to SBUF, DMA it to partition 0
                # (the custom DVE/GpSimd ops operate on absolute partition
                # 0 in hardware), approx-reciprocal, partition-broadcast on
                # the idle GpSimd engine, one wide DVE multiply, scatter.
                den = small.tile([P, 2 * QBLK], F32, tag="den")
                nc.vector.tensor_copy(den[DK:DA, :], C[DK:DA, :])
                den0 = small.tile([1, 2 * QBLK], F32, tag="den0")
                nc.sync.dma_start(den0[0:1, :], den[DK:DA, :])
                rec = small.tile([1, 2 * QBLK], F32, tag="rec")
                nc.vector.reciprocal_approx_fast(rec[0:1, :], den0[0:1, :])
                rb_sb = small.tile([P, 2 * QBLK], F32, tag="rb_sb")
                nc.gpsimd.partition_broadcast(rb_sb[0:DK, :], rec[0:1, :],
                                              channels=DK)
                ctx_sb = small.tile([P, 2 * QBLK], BF16, tag="ctx_sb")
                nc.vector.tensor_mul(ctx_sb[0:DK, :], C[0:DK, :],
                                     rb_sb[0:DK, :])
                # scatter both heads: dst core d = (g%2)*4 + s owns tokens
                # [d*TPB, (d+1)*TPB) of chunk g//2.
                chunk = g // 2
                dsl_s = ts(g % 2, QBLK // TPB)
                nc.sync.dma_start(
                    a2a_in[chunk][dsl_s, 0:DK].rearrange("s p t -> p s t"),
                    ctx_sb[0:DK, 0:QBLK].rearrange("p (s t) -> p s t",
                                                   s=QBLK // TPB))
                nc.sync.dma_start(
                    a2a_in[chunk][dsl_s, DK:P].rearrange("s p t -> p s t"),
                    ctx_sb[0:DK, QBLK:].rearrange("p (s t) -> p s t",
                                                  s=QBLK // TPB))
                # leftover fillers after the chain is in flight
                while done < nfill:
                    fill[done]()
                    done += 1

            # ---- interleave plan ---------------------------------------
            # fillers[g] = PE work units emitted inside q-block g's k-loop
            fillers = [[] for _ in range(G)]

            # proj pairs >=1: 6 slab-proj units each, placed in the two
            # q-blocks before the first q-block that touches their tokens
            for p_ in range(1, NPAIR):
                toks0 = p_ * 2 * QBLK
                b_p = toks0 // T
                qi_first = (toks0 % T) // QBLK
                g_need = b_p * NQB + qi_first
                span = [gg for gg in (g_need - 2, g_need - 1) if gg >= 0]
                units = []
                # slab-major so the first span q-block only needs slab 2p
                for s in (2 * p_, 2 * p_ + 1):
                    for w_sb, b_sb, dst in ((wq_sb, bq_sb, qT),
                                            (wk_sb, bk_sb, kT),
                                            (wv_sb, bv_sb, vT)):
                        units.append(
                            lambda w=w_sb, bb=b_sb, dd=dst, ss=s:
                            proj_slab(w, bb, dd, ss))
                if len(span) == 1:
                    fillers[span[0]] += units
                else:
                    fillers[span[0]] += units[:3]
                    fillers[span[1]] += units[3:]

            # v_nat blocks: j needed by q-block (b, jloc//KPQ); emit one
            # q-block earlier (prologue covers the first KPQ blocks)
            vnat_pro = []
            for j in range(NJ):
                b_j = j // JPB
                g_need = b_j * NQB + (j % JPB) // KPQ
                g_slot = g_need - 1
                if g_slot < 0:
                    vnat_pro.append(j)
                else:
                    fillers[g_slot].append(lambda jj=j: v_nat_block(jj))

            # output projections: op p consumes chunk p, whose collective
            # is triggered at the end of q-block 2p+1.  Collectives take
            # 20-50us to COMPLETE (barrier + serialized CC stream), and a
            # gather emitted too early head-blocks the sync queue (the
            # scheduler hoists it ahead of later scatters) -- so only
            # schedule an op as filler with >= 4 q-blocks of soak.
            ctxg_tiles = {}

            def outproj_compute(p):
                ctxg = ctxg_tiles.pop(p, None)
                if ctxg is None:
                    ctxg = outproj_gather(p)
                outproj_half(p, ctxg, 0)
                outproj_half(p, ctxg, 1)

            tail_ps = []
            for p_ in range(NOP):
                g_slot = 2 * p_ + 6
                if g_slot < G:
                    fillers[g_slot].append(lambda pp=p_: outproj_compute(pp))
                else:
                    tail_ps.append(p_)
            # tail ops: pre-issue each gather right after the NEXT chunk's
            # doorbell (its own collective completed a full chunk ago), so
            # the tail's sync queue only ever waits on the LAST collective
            pregather = {p_ + 1: p_ for p_ in tail_ps if p_ + 1 < NCHUNK}

            # later x slabs + the wide tail weights on gpsimd, staggered
            # so their buffer-free waits never delay a chunk's broadcast
            gp_bulk = {}
            for s in range(4, NSLAB):
                gp_bulk.setdefault(s - 4, []).append(
                    lambda ss=s: load_slab(ss, nc.gpsimd))
            gp_bulk.setdefault(min(1, G - 2), []).append(load_wo)

            # ---- emission ----------------------------------------------
            proj_pair(0)
            for j in vnat_pro:
                v_nat_block(j)

            g = 0
            for b in range(B):
                for qi in range(NQB):
                    attention_qblock(b, qi, fillers[g])
                    if g % 2 == 1:
                        k = g // 2
                        collective(k)
                        if k in pregather:
                            ctxg_tiles[pregather[k]] = outproj_gather(
                                pregather[k])
                    for fn in gp_bulk.get(g, []):
                        fn()
                    g += 1

            # tail: the last chunk's collective is in flight; its input is
            # pre-gathered for all but the last op, so the reserved output
            # projections + a short warmup cover the collective latency
            assert tail_ps
            for p_ in tail_ps[:-1]:
                outproj_compute(p_)
            warmup(10, moving=qT[:, 0:QBLK])
            outproj_compute(tail_ps[-1])

    nc.compile()
    return nc


_NC_CACHE = {}


def _get_nc(B, T):
    key = (B, T)
    if key not in _NC_CACHE:
        _NC_CACHE[key] = build_nc(B, T)
    return _NC_CACHE[key]


def make_in_maps(x, Wq, bq, Wk, bk, Wv, bv, Wo, bo):
    B, T, _ = x.shape
    NTOK = B * T
    NSLAB = NTOK // QBLK
    KO = D // P
    x = np.asarray(x, np.float32)
    # [D, NTOK] -> [p, slab, ko, t]: one contiguous 8KB DMA descriptor per
    # partition per token slab.
    xT = x.reshape(NTOK, D).T  # [D, NTOK]
    xT_t = np.ascontiguousarray(
        xT.reshape(KO, P, NSLAB, QBLK).transpose(1, 2, 0, 3)).astype(NP_BF16)

    def wtile(W):
        # [D, M] -> [p, ko, m] so each partition's row is contiguous
        wt = np.asarray(W, np.float32)
        return np.ascontiguousarray(
            wt.reshape(KO, P, -1).transpose(1, 0, 2)).astype(NP_BF16)

    woT = wtile(np.asarray(Wo, np.float32).T)
    bo = np.asarray(bo, np.float32)
    # 128-wide causal 0/1 triangle for the diagonal partial columns
    keep = np.arange(KBLK)[None, :] >= np.arange(P)[:, None]
    mask = np.where(keep, 1.0, 0.0).astype(NP_BF16)
    ident = np.eye(P, dtype=NP_BF16)
    ones = np.ones((P, NTOK // P), NP_BF16)
    in_maps = []
    for c in range(NCORES):
        sl = slice(DSL * c, DSL * (c + 1))
        in_maps.append({
            "xT": xT_t,
            "wqT": wtile(np.asarray(Wq, np.float32)[sl].T * 0.125),
            "wkT": wtile(np.asarray(Wk, np.float32)[sl].T),
            "wvT": wtile(np.asarray(Wv, np.float32)[sl].T),
            "woT": woT,
            "bq": (np.asarray(bq, np.float32)[sl] * 0.125).reshape(DSL, 1),
            "bk": np.asarray(bk, np.float32)[sl].reshape(DSL, 1),
            "bv": np.asarray(bv, np.float32)[sl].reshape(DSL, 1),
            "bo": bo,
            "mask": mask,
            "ident": ident,
            "identr": np.eye(P, dtype=np.float32),
            "ones": ones,
        })
    return in_maps


def unshard(res_c, B, T):
    """res_c: [NCORES, NCHUNK*128, D] core-major outputs -> [B, T, D].

    Core c's rows are [chunk, 128] with chunk k covering tokens
    [k*1024 + c*128, k*1024 + (c+1)*128)."""
    NCHUNK = B * (T // QBLK) // 2
    TPB = 2 * QBLK // NCORES
    out = res_c.reshape(NCORES, NCHUNK, TPB, D).transpose(1, 0, 2, 3)
    return np.ascontiguousarray(out.reshape(B, T, D))


LAST_RESULTS = None


def kernel(x, Wq, bq, Wk, bk, Wv, bv, Wo, bo, trace=False, trace_cores=None):
    global LAST_RESULTS
    B, T, _ = x.shape
    nc = _get_nc(B, T)
    in_maps = make_in_maps(x, Wq, bq, Wk, bk, Wv, bv, Wo, bo)
    kw = {}
    if trace:
        kw = dict(trace=True, trace_cores=trace_cores)
    res = bass_utils.run_bass_kernel_spmd(nc, in_maps,
                                          core_ids=list(range(NCORES)), **kw)
    LAST_RESULTS = res
    res_c = np.stack([res.results[c]["out"] for c in range(NCORES)], axis=0)
    return unshard(res_c, B, T)
